# revision 6
# baseline (speedup 1.0000x reference)
"""Trainium2 Bass kernel for nn_DifferentiableQUBO.

reference:
    decisions = sigmoid(scores)            # elementwise, N = 16_777_216 f32
    qubo_loss = sum(decisions * decisions) # scalar
    returns (decisions, qubo_loss)

Sharding: scores split into 8 contiguous shards of 2_097_152 f32 (data
parallel, one per NeuronCore).  Each core returns its decisions shard and
NT=8 partial sums of squares; the host concatenates the shards and adds
the 8*NT partials (f64) into the scalar loss.

Per-core kernel (raw bass, hand-placed semaphores — measured 52.2 us vs
68.6 us for the TileContext version of the same dataflow):
  - whole 8 MiB shard is SBUF-resident as NT=8 tiles of [128, 2048]
  - SP issues all 8 loads back-to-back (HBM stays saturated), then the
    stores, each gated on its sigmoid's completion semaphore
  - ACT runs sigmoid(tile i) paced by load i, with Square+accum_out
    (per-partition sum of squares into acc[:, i]) interleaved lag-2 into
    the load-pacing gaps
  - the idle PE reduces acc across partitions (ones.T @ acc -> psum[1,8])
    so the loss leaves as ONE 32-byte store from partition 0 (a [128,1]
    store = 128 4-byte descriptors measured ~7 us of completion latency)
  - DVE/GpSimd unused

Known-broken on this runtime (bisected on HW): DVE TENSOR_TENSOR_REDUCE
(walrus ISA-length error), DVE scalar_tensor_tensor w/ accum
(NRT_EXEC_UNIT_UNRECOVERABLE).  Sum-of-squares therefore runs on ACT.
"""

import contextlib
import os
import sys

import numpy as np

for _p in ("/opt/trn_rl_repo", "/root/.axon_site/_ro/trn_rl_repo"):
    if os.path.isdir(_p) and _p not in sys.path:
        sys.path.insert(0, _p)
        break

import concourse.bacc as bacc
import concourse.mybir as mybir
from concourse.bass_utils import run_bass_kernel_spmd

N = 16_777_216
NCORES = 8
SHARD = N // NCORES  # 2_097_152
P = 128
F = 2048
NT = SHARD // (P * F)  # 8
F32 = mybir.dt.float32

_nc_cache = None


def _build():
    nc = bacc.Bacc("TRN2", num_devices=NCORES)
    x = nc.dram_tensor("scores", [SHARD], F32, kind="ExternalInput")
    d = nc.dram_tensor("decisions", [SHARD], F32, kind="ExternalOutput")
    partial = nc.dram_tensor("partial", [1, NT], F32, kind="ExternalOutput")
    x3 = x.rearrange("(n p f) -> n p f", p=P, f=F)
    d3 = d.rearrange("(n p f) -> n p f", p=P, f=F)

    with contextlib.ExitStack() as stack:
        ec = stack.enter_context
        xts = [ec(nc.sbuf_tensor(f"xt{j}", [P, F], F32)) for j in range(NT)]
        dts = [ec(nc.sbuf_tensor(f"dt{j}", [P, F], F32)) for j in range(NT)]
        acc = ec(nc.sbuf_tensor("acc", [P, NT], F32))
        accflat = ec(nc.sbuf_tensor("accflat", [1, NT], F32))
        psum = ec(nc.psum_tensor([1, NT], F32))
        load_sems = [ec(nc.semaphore(f"load_sem{j}")) for j in range(NT)]
        store_sem = ec(nc.semaphore("store_sem"))
        part_sem = ec(nc.semaphore("part_sem"))
        sig_sem = ec(nc.semaphore("sig_sem"))
        sq_sem = ec(nc.semaphore("sq_sem"))
        pe_sem = ec(nc.semaphore("pe_sem"))
        fin_sem = ec(nc.semaphore("fin_sem"))
        block = ec(nc.Block(no_gpsimd_drain=True))

        @block.sync
        def _(sync):
            for i in range(NT):
                sync.dma_start(out=xts[i][:], in_=x3[i]).then_inc(load_sems[i], 16)
            for i in range(NT):
                sync.wait_ge(sig_sem, i + 1)
                sync.dma_start(out=d3[i], in_=dts[i][:]).then_inc(store_sem, 16)
            sync.wait_ge(fin_sem, 1)
            sync.dma_start(out=partial[:], in_=accflat[:]).then_inc(part_sem, 16)
            sync.wait_ge(store_sem, 16 * NT)
            sync.wait_ge(part_sem, 16)

        @block.scalar
        def _(scalar):
            def sig(i):
                scalar.wait_ge(load_sems[i], 16)
                nc.scalar.activation(
                    out=dts[i][:],
                    in_=xts[i][:],
                    func=mybir.ActivationFunctionType.Sigmoid,
                ).then_inc(sig_sem, 1)

            def sq(i):
                # satisfied at issue time (sigmoids run 2 ahead); formally
                # closes the same-engine pipeline RAW on dt[i]
                scalar.wait_ge(sig_sem, i + 1)
                nc.scalar.activation(
                    out=xts[i][:],
                    in_=dts[i][:],
                    func=mybir.ActivationFunctionType.Square,
                    accum_out=acc[:, i : i + 1],
                ).then_inc(sq_sem, 1)

            sig(0)
            if NT > 1:
                sig(1)
            for i in range(NT):
                if i + 2 < NT:
                    sig(i + 2)
                sq(i)
            scalar.wait_ge(pe_sem, 1)
            nc.scalar.copy(out=accflat[:], in_=psum[:]).then_inc(fin_sem, 1)

        @block.tensor
        def _(tensor):
            tensor.wait_ge(sq_sem, NT)
            ones = nc.const_aps.tensor(1.0, (P, 1))
            nc.tensor.matmul(
                out=psum[:], lhsT=ones, rhs=acc[:], start=True, stop=True
            ).then_inc(pe_sem, 1)

    nc.finalize()
    return nc


def _get_nc():
    global _nc_cache
    if _nc_cache is None:
        _nc_cache = _build()
    return _nc_cache


def run(scores: np.ndarray, trace: bool = False):
    """Run on 8 cores. Returns (decisions, loss, exec_time_ns|None)."""
    scores = np.ascontiguousarray(np.asarray(scores, dtype=np.float32))
    assert scores.shape == (N,), scores.shape
    shards = scores.reshape(NCORES, SHARD)
    in_maps = [{"scores": shards[c]} for c in range(NCORES)]
    res = run_bass_kernel_spmd(
        _get_nc(), in_maps, core_ids=list(range(NCORES)), trace=trace
    )
    decisions = np.concatenate(
        [np.asarray(res.results[c]["decisions"]) for c in range(NCORES)]
    )
    partials = np.stack([np.asarray(res.results[c]["partial"]) for c in range(NCORES)])
    loss = np.float32(partials.astype(np.float64).sum())
    return decisions, loss, res.exec_time_ns


def kernel(scores: np.ndarray, data: np.ndarray = None, **_unused) -> tuple:
    decisions, loss, _ = run(scores, trace=False)
    return decisions, loss
